# revision 2
# baseline (speedup 1.0000x reference)
"""LIF activation scan kernel for Trainium2, SPMD over 8 NeuronCores.

Computation (per element, T=4 scan over leading dim):
    m = 0.25*m + x_t;  s_t = (m > 0.5);  m = m*(1-s_t)
with m0 = 0. Output is the spike train s (float32 0/1), shape [4,64,128,32,32].

Sharding: batch dim (axis 1, size 64) split 8 ways -> per-core x shard
[4, 8, 128, 32, 32] = [4, 128, 8192] contiguous f32 (16 MiB in, 16 MiB out).

Per-step DVE ops (all fp32 SBUF):
    m   = mq + x_t          tensor_tensor add   (1x mode)
    s_t = (m > 0.5)         tensor_scalar is_gt (2x mode)
    nsq = (m<=0.5)*0.25     tensor_scalar fused two-op (2x mode)
    mq  = m * nsq           tensor_tensor mult  (1x mode)
mq is 0.25*(membrane after reset), so the next add needs no extra scale.
t=0 shortcut: m == x_0, so the first add is skipped. t=3 skips nsq/mq.
Exactness: mask mult and *0.25 are exact in f32, so the membrane trajectory
is bit-identical to the reference's (m*(1-s))*0.25 + x ordering.
"""

import numpy as np

N_CORES = 8
T = 4
B, C, H, W = 64, 128, 32, 32
BS = B // N_CORES  # 8 batches per core
P = 128
FD = BS * C * H * W // P  # 8192 free elems per partition per timestep
CHUNK = 1024
N_CHUNKS = FD // CHUNK

_CACHE = {}


def _build_program(reps: int = 1):
    import concourse.bacc as bacc
    import concourse.tile as tile
    import concourse.mybir as mybir

    f32 = mybir.dt.float32
    Alu = mybir.AluOpType

    nc = bacc.Bacc("TRN2", target_bir_lowering=False, debug=False,
                   num_devices=N_CORES)
    x_t = nc.dram_tensor("x", [T, P, FD], f32, kind="ExternalInput")
    out_t = nc.dram_tensor("out", [T, P, FD], f32, kind="ExternalOutput")
    x_ap = x_t.ap()
    out_ap = out_t.ap()

    with tile.TileContext(nc) as tc:
        with (
            tc.tile_pool(name="xp", bufs=3) as xp,
            tc.tile_pool(name="sp", bufs=3) as sp,
            tc.tile_pool(name="wp", bufs=4) as wp,
        ):
            for c in range(N_CHUNKS * reps):
                c = c % N_CHUNKS
                sl = slice(c * CHUNK, (c + 1) * CHUNK)
                xs = []
                for t in range(T):
                    xt = xp.tile([P, CHUNK], f32, tag=f"x{t}")
                    nc.sync.dma_start(xt[:], x_ap[t, :, sl])
                    xs.append(xt)

                # t = 0: membrane is exactly x_0
                s0 = sp.tile([P, CHUNK], f32, tag="s0")
                nc.vector.tensor_scalar(s0[:], xs[0][:], 0.5, None, Alu.is_gt)
                nc.sync.dma_start(out_ap[0, :, sl], s0[:])
                nsq = wp.tile([P, CHUNK], f32, tag="nsq")
                nc.vector.tensor_scalar(nsq[:], xs[0][:], 0.5, 0.25,
                                        Alu.is_le, Alu.mult)
                mq = wp.tile([P, CHUNK], f32, tag="mq")
                nc.vector.tensor_tensor(mq[:], xs[0][:], nsq[:], Alu.mult)

                for t in range(1, T):
                    m = wp.tile([P, CHUNK], f32, tag="m")
                    nc.vector.tensor_tensor(m[:], mq[:], xs[t][:], Alu.add)
                    st = sp.tile([P, CHUNK], f32, tag=f"s{t}")
                    nc.vector.tensor_scalar(st[:], m[:], 0.5, None, Alu.is_gt)
                    nc.sync.dma_start(out_ap[t, :, sl], st[:])
                    if t < T - 1:
                        nsq = wp.tile([P, CHUNK], f32, tag="nsq")
                        nc.vector.tensor_scalar(nsq[:], m[:], 0.5, 0.25,
                                                Alu.is_le, Alu.mult)
                        mq = wp.tile([P, CHUNK], f32, tag="mq")
                        nc.vector.tensor_tensor(mq[:], m[:], nsq[:], Alu.mult)

    nc.compile()
    return nc


def _get_program():
    if "nc" not in _CACHE:
        _CACHE["nc"] = _build_program()
    return _CACHE["nc"]


def kernel(x: np.ndarray, _trace: bool = False, _trace_kwargs: dict | None = None):
    from concourse.bass_utils import run_bass_kernel_spmd

    assert x.shape == (T, B, C, H, W) and x.dtype == np.float32
    nc = _get_program()

    in_maps = []
    for i in range(N_CORES):
        shard = np.ascontiguousarray(x[:, i * BS:(i + 1) * BS])
        in_maps.append({"x": shard.reshape(T, P, FD)})

    res = run_bass_kernel_spmd(
        nc, in_maps, core_ids=list(range(N_CORES)),
        trace=_trace, **(_trace_kwargs or {}),
    )

    out = np.empty((T, B, C, H, W), dtype=np.float32)
    for i in range(N_CORES):
        out[:, i * BS:(i + 1) * BS] = (
            res.results[i]["out"].reshape(T, BS, C, H, W)
        )
    if _trace:
        return out, res
    return out


# revision 7
# speedup vs baseline: 1.4145x; 1.4145x over previous
"""LIF activation scan kernel for Trainium2, SPMD over 8 NeuronCores.

Computation (per element, T=4 scan over leading dim):
    m = 0.25*m + x_t;  s_t = (m > 0.5);  m = m*(1-s_t)
with m0 = 0. Output is the spike train s (float32 0/1), shape [4,64,128,32,32].

Sharding: batch dim (axis 1, size 64) split 8 ways -> per-core x shard
[4, 8, 128, 32, 32] = [4, 128, 8192] contiguous f32 (16 MiB in, 16 MiB out).

Per-step DVE ops (all fp32 SBUF):
    m   = mq + x_t          tensor_tensor add   (1x mode)
    s_t = (m > 0.5)         tensor_scalar is_gt (2x mode)
    nsq = (m<=0.5)*0.25     tensor_scalar fused two-op (2x mode)
    mq  = m * nsq           tensor_tensor mult  (1x mode)
mq is 0.25*(membrane after reset), so the next add needs no extra scale.
t=0 shortcut: m == x_0, so the first add is skipped. t=3 skips nsq/mq.
Exactness: mask mult and *0.25 are exact in f32, so the membrane trajectory
is bit-identical to the reference's (m*(1-s))*0.25 + x ordering.
"""

import numpy as np

N_CORES = 8
T = 4
B, C, H, W = 64, 128, 32, 32
BS = B // N_CORES  # 8 batches per core
P = 128
FD = BS * C * H * W // P  # 8192 free elems per partition per timestep
CHUNK = 2048
N_CHUNKS = FD // CHUNK

_CACHE = {}


def _build_program(reps: int = 1):
    import concourse.bacc as bacc
    import concourse.tile as tile
    import concourse.mybir as mybir

    f32 = mybir.dt.float32
    bf16 = mybir.dt.bfloat16
    Alu = mybir.AluOpType
    Act = mybir.ActivationFunctionType

    nc = bacc.Bacc("TRN2", target_bir_lowering=False, debug=False,
                   num_devices=N_CORES)
    x_t = nc.dram_tensor("x", [T, P, FD], f32, kind="ExternalInput")
    # Spikes leave the device as bf16 Sign(m-0.5) in {-1, 0, 1}; the host
    # gather applies max(s, 0) and the f32 cast (both exact on {-1,0,1}).
    out_t = nc.dram_tensor("out", [T, P, FD], bf16, kind="ExternalOutput")
    x_ap = x_t.ap()
    out_ap = out_t.ap()

    with tile.TileContext(nc) as tc:
        with (
            tc.tile_pool(name="cp", bufs=1) as cp,
            tc.tile_pool(name="xp", bufs=2) as xp,
            tc.tile_pool(name="sp", bufs=2) as sp,
            tc.tile_pool(name="wp", bufs=2) as wp,
        ):
            neg_half = cp.tile([P, 1], f32)
            nc.vector.memset(neg_half[:], -0.5)
            for c in range(N_CHUNKS * reps):
                c = c % N_CHUNKS
                sl = slice(c * CHUNK, (c + 1) * CHUNK)
                xs = []
                for t in range(T):
                    xt = xp.tile([P, CHUNK], f32, tag=f"x{t}")
                    nc.sync.dma_start(xt[:], x_ap[t, :, sl])
                    xs.append(xt)

                # t = 0: membrane is exactly x_0
                s0 = sp.tile([P, CHUNK], bf16, tag="s0")
                nc.scalar.activation(s0[:], xs[0][:], Act.Sign, bias=neg_half[:])
                nc.sync.dma_start(out_ap[0, :, sl], s0[:])
                nsq = wp.tile([P, CHUNK], f32, tag="nsq")
                nc.vector.tensor_scalar(nsq[:], xs[0][:], 0.5, 0.25,
                                        Alu.is_le, Alu.mult)
                mq = wp.tile([P, CHUNK], f32, tag="mq")
                nc.vector.tensor_tensor(mq[:], xs[0][:], nsq[:], Alu.mult)

                for t in range(1, T):
                    m = wp.tile([P, CHUNK], f32, tag="m")
                    nc.vector.tensor_tensor(m[:], mq[:], xs[t][:], Alu.add)
                    st = sp.tile([P, CHUNK], bf16, tag=f"s{t}")
                    nc.scalar.activation(st[:], m[:], Act.Sign, bias=neg_half[:])
                    nc.sync.dma_start(out_ap[t, :, sl], st[:])
                    if t < T - 1:
                        nsq = wp.tile([P, CHUNK], f32, tag="nsq")
                        nc.vector.tensor_scalar(nsq[:], m[:], 0.5, 0.25,
                                                Alu.is_le, Alu.mult)
                        mq = wp.tile([P, CHUNK], f32, tag="mq")
                        nc.vector.tensor_tensor(mq[:], m[:], nsq[:], Alu.mult)

    nc.compile()
    return nc


def _get_program():
    if "nc" not in _CACHE:
        _CACHE["nc"] = _build_program()
    return _CACHE["nc"]


def kernel(x: np.ndarray, _trace: bool = False, _trace_kwargs: dict | None = None):
    from concourse.bass_utils import run_bass_kernel_spmd

    assert x.shape == (T, B, C, H, W) and x.dtype == np.float32
    nc = _get_program()

    in_maps = []
    for i in range(N_CORES):
        shard = np.ascontiguousarray(x[:, i * BS:(i + 1) * BS])
        in_maps.append({"x": shard.reshape(T, P, FD)})

    res = run_bass_kernel_spmd(
        nc, in_maps, core_ids=list(range(N_CORES)),
        trace=_trace, **(_trace_kwargs or {}),
    )

    out = np.empty((T, B, C, H, W), dtype=np.float32)
    for i in range(N_CORES):
        s = res.results[i]["out"].astype(np.float32)
        np.maximum(s, 0.0, out=s)
        out[:, i * BS:(i + 1) * BS] = s.reshape(T, BS, C, H, W)
    if _trace:
        return out, res
    return out


# revision 8
# speedup vs baseline: 1.5628x; 1.1049x over previous
"""LIF activation scan kernel for Trainium2, SPMD over 8 NeuronCores.

Computation (per element, T=4 scan over leading dim):
    m = 0.25*m + x_t;  s_t = (m > 0.5);  m = m*(1-s_t)
with m0 = 0. Output is the spike train s (float32 0/1), shape [4,64,128,32,32].

Sharding: batch dim (axis 1, size 64) split 8 ways -> per-core x shard
[4, 8, 128, 32, 32] = [4, 128, 8192] contiguous f32 (16 MiB in, 16 MiB out).

Per-step DVE ops (all fp32 SBUF):
    m   = mq + x_t          tensor_tensor add   (1x mode)
    s_t = (m > 0.5)         tensor_scalar is_gt (2x mode)
    nsq = (m<=0.5)*0.25     tensor_scalar fused two-op (2x mode)
    mq  = m * nsq           tensor_tensor mult  (1x mode)
mq is 0.25*(membrane after reset), so the next add needs no extra scale.
t=0 shortcut: m == x_0, so the first add is skipped. t=3 skips nsq/mq.
Exactness: mask mult and *0.25 are exact in f32, so the membrane trajectory
is bit-identical to the reference's (m*(1-s))*0.25 + x ordering.
"""

import numpy as np

N_CORES = 8
T = 4
B, C, H, W = 64, 128, 32, 32
BS = B // N_CORES  # 8 batches per core
P = 128
FD = BS * C * H * W // P  # 8192 free elems per partition per timestep
CHUNK = 2048
N_CHUNKS = FD // CHUNK

_CACHE = {}


def _build_program(reps: int = 1):
    import concourse.bacc as bacc
    import concourse.tile as tile
    import concourse.mybir as mybir

    f32 = mybir.dt.float32
    bf16 = mybir.dt.bfloat16
    Alu = mybir.AluOpType
    Act = mybir.ActivationFunctionType

    nc = bacc.Bacc("TRN2", target_bir_lowering=False, debug=False,
                   num_devices=N_CORES)
    x_t = nc.dram_tensor("x", [T, P, FD], f32, kind="ExternalInput")
    # Spikes leave the device as uint8 Sign(m-0.5): 1 on spike; 0 or 255
    # (wrapped -1) otherwise. The host gather maps (s == 1) -> 1.0f.
    out_t = nc.dram_tensor("out", [T, P, FD], mybir.dt.uint8,
                           kind="ExternalOutput")
    x_ap = x_t.ap()
    out_ap = out_t.ap()

    with tile.TileContext(nc) as tc:
        with (
            tc.tile_pool(name="cp", bufs=1) as cp,
            tc.tile_pool(name="xp", bufs=3) as xp,
            tc.tile_pool(name="sp", bufs=2) as sp,
            tc.tile_pool(name="wp", bufs=2) as wp,
        ):
            neg_half = cp.tile([P, 1], f32)
            nc.vector.memset(neg_half[:], -0.5)
            for c in range(N_CHUNKS * reps):
                c = c % N_CHUNKS
                sl = slice(c * CHUNK, (c + 1) * CHUNK)
                xs = []
                for t in range(T):
                    xt = xp.tile([P, CHUNK], f32, tag=f"x{t}")
                    nc.sync.dma_start(xt[:], x_ap[t, :, sl])
                    xs.append(xt)

                # t = 0: membrane is exactly x_0
                s0 = sp.tile([P, CHUNK], mybir.dt.uint8, tag="s0")
                nc.scalar.activation(s0[:], xs[0][:], Act.Sign, bias=neg_half[:])
                nc.sync.dma_start(out_ap[0, :, sl], s0[:])
                nsq = wp.tile([P, CHUNK], f32, tag="nsq")
                nc.vector.tensor_scalar(nsq[:], xs[0][:], 0.5, 0.25,
                                        Alu.is_le, Alu.mult)
                mq = wp.tile([P, CHUNK], f32, tag="mq")
                nc.vector.tensor_tensor(mq[:], xs[0][:], nsq[:], Alu.mult)

                for t in range(1, T):
                    m = wp.tile([P, CHUNK], f32, tag="m")
                    nc.vector.tensor_tensor(m[:], mq[:], xs[t][:], Alu.add)
                    st = sp.tile([P, CHUNK], mybir.dt.uint8, tag=f"s{t}")
                    nc.scalar.activation(st[:], m[:], Act.Sign, bias=neg_half[:])
                    nc.sync.dma_start(out_ap[t, :, sl], st[:])
                    if t < T - 1:
                        nsq = wp.tile([P, CHUNK], f32, tag="nsq")
                        nc.vector.tensor_scalar(nsq[:], m[:], 0.5, 0.25,
                                                Alu.is_le, Alu.mult)
                        mq = wp.tile([P, CHUNK], f32, tag="mq")
                        nc.vector.tensor_tensor(mq[:], m[:], nsq[:], Alu.mult)

    nc.compile()
    return nc


def _get_program():
    if "nc" not in _CACHE:
        _CACHE["nc"] = _build_program()
    return _CACHE["nc"]


def kernel(x: np.ndarray, _trace: bool = False, _trace_kwargs: dict | None = None):
    from concourse.bass_utils import run_bass_kernel_spmd

    assert x.shape == (T, B, C, H, W) and x.dtype == np.float32
    nc = _get_program()

    in_maps = []
    for i in range(N_CORES):
        shard = np.ascontiguousarray(x[:, i * BS:(i + 1) * BS])
        in_maps.append({"x": shard.reshape(T, P, FD)})

    res = run_bass_kernel_spmd(
        nc, in_maps, core_ids=list(range(N_CORES)),
        trace=_trace, **(_trace_kwargs or {}),
    )

    out = np.empty((T, B, C, H, W), dtype=np.float32)
    for i in range(N_CORES):
        s = (res.results[i]["out"] == 1).astype(np.float32)
        out[:, i * BS:(i + 1) * BS] = s.reshape(T, BS, C, H, W)
    if _trace:
        return out, res
    return out
